# revision 36
# baseline (speedup 1.0000x reference)
"""HetConv (3x3 block-diagonal-by-residue + 1x1 elsewhere) on 8 trn2 cores.

Strategy: data-parallel over batch (4 images/core, weights replicated).
Per core: implicit-GEMM conv over a 66x66 zero-padded SBUF image with
channels permuted by residue mod 4 (done via strided DMA access patterns,
never materialized on host). Effective weight packs into 20 [128x128]
bf16 matmul slots per spatial tile instead of 36 dense ones, in
consumption order:
  - slots 10c+ti, ti in 0..8: tap (ky,kx)=divmod(ti,3), block-diag Wk for
    groups (2c, 2c+1); center tap also carries same-chunk W1 off-diagonal.
  - slot 10c+9: cross-chunk center-tap W1 (other ic chunk -> oc chunk c).

DMA schedule is latency-tuned for the serial DMA transfer engine: inputs
stream in band-sized row strips (image 0's chunk 1 rides the ACT/HWDGE
queue interleaved with the weight DMAs; everything else on gpsimd/SWDGE)
so the PE never starves at the start or at image boundaries; outputs go
out full-width, one DMA per (band, oc chunk), split across the two HWDGE
queues; all operands are bf16 (error ~3e-3, gate 2e-2), halving DMA
traffic, with fp32 PSUM accumulation.
"""
import sys

sys.path.insert(0, "/opt/trn_rl_repo")

import numpy as np
import concourse.bacc as bacc
import concourse.mybir as mybir
from concourse import tile
from concourse.bass_utils import run_bass_kernel_spmd

N_CORES = 8
B, C, H, W = 32, 256, 64, 64
BP = B // N_CORES          # images per core
HP, WP = H + 2, W + 2      # padded image
NTILES = 8                 # output row-bands per image
RPT = H // NTILES          # rows per band
NFREE = RPT * W            # matmul moving free size (512)
NSLOTS = 20

_PROG = None


def _build(reps=1):
    nc = bacc.Bacc("TRN2", target_bir_lowering=False, debug=False,
                   num_devices=N_CORES)
    f32 = mybir.dt.float32
    bf16 = mybir.dt.bfloat16

    # x arrives host-padded to [BP, C, 66, 66] (zero border), pre-cast to
    # bf16 (error ~3e-3 vs the 2e-2 gate; halves all DMA traffic).
    x = nc.dram_tensor("x", [BP, C, HP, WP], bf16, kind="ExternalInput").ap()
    w = nc.dram_tensor("w", [128, NSLOTS * 128], bf16,
                       kind="ExternalInput").ap()
    out = nc.dram_tensor("out", [BP, C, H, W], bf16,
                         kind="ExternalOutput").ap()

    # channel c = 4k + g  ->  [b, g, k, ...]
    x_r = x.rearrange("b (k four) h w -> b four k h w", four=4)
    out_r = out.rearrange("b (k four) h w -> b four k h w", four=4)

    # input row strips: band 0 needs padded rows 0..9; band nt (>=1) needs
    # rows 8nt..8nt+9 = tail of strip nt-1 plus strip nt
    strips = [(0, 10)] + [(8 * k + 2, 8) for k in range(1, NTILES)]

    with tile.TileContext(nc) as tc:
        with (
            tc.tile_pool(name="wpool", bufs=1) as wpool,
            tc.tile_pool(name="xpool", bufs=2) as xpool,
            tc.tile_pool(name="opool", bufs=4) as opool,
            tc.tile_pool(name="pspool", bufs=4, space="PSUM") as pspool,
        ):
            wt = wpool.tile([128, NSLOTS * 128], bf16)

            def wdma(eng, lo, hi):
                eng.dma_start(out=wt[:, lo * 128:hi * 128],
                              in_=w[:, lo * 128:hi * 128])

            # weight DMAs in slot-consumption order: slots 0-4 on SP so the
            # whole first tile batch is ready together (the cold-start batch
            # is charged at full speed), the rest on ACT interleaved with
            # image 0's chunk-1 strips (emitted in the loop below)
            wdma(nc.sync, 0, 3)
            wdma(nc.sync, 3, 5)
            wdma(nc.sync, 5, 10)

            def wslot(s):
                return wt[:, s * 128:(s + 1) * 128]

            imgs = [i % BP for i in range(BP * reps)]
            for ii, img in enumerate(imgs):
                xps = [xpool.tile([128, HP * WP], bf16, tag=f"xp{c}",
                                  name=f"xp{c}")
                       for c in (0, 1)]
                # band-interleaved strip DMAs; partitions 0-63 <- residue 2c,
                # 64-127 <- 2c+1, one contiguous run per partition per strip.
                def sdma(eng, cchunk, r0, nr):
                    eng.dma_start(
                        out=xps[cchunk][:, r0 * WP:(r0 + nr) * WP],
                        in_=x_r[img, 2 * cchunk:2 * cchunk + 2, :,
                                r0:r0 + nr, :],
                    )

                if ii == 0:
                    # image 0: chunk 0 on Pool, chunk 1 on ACT interleaved
                    # with the remaining weight DMAs, so both chunks of band
                    # 0 land in parallel and the PE ramps without starving
                    sdma(nc.scalar, 1, 0, 10)
                    wdma(nc.scalar, 10, 15)
                    wdma(nc.scalar, 15, 20)
                    sdma(nc.gpsimd, 0, 0, 8)
                    sdma(nc.gpsimd, 0, 8, 2)
                    for r0, nr in strips[1:]:
                        sdma(nc.gpsimd, 0, r0, nr)
                        sdma(nc.scalar, 1, r0, nr)
                else:
                    # later images stream on Pool, whose SWDGE pipeline has
                    # plenty of headroom once compute is deep
                    for r0, nr in strips:
                        for cchunk in (0, 1):
                            sdma(nc.gpsimd, cchunk, r0, nr)
                xvs = [xp[:, :].rearrange("p (h w) -> p h w", w=WP)
                       for xp in xps]

                for nt in range(NTILES):
                    def rhs(cchunk, ky, kx):
                        return xvs[cchunk][:, nt * RPT + ky:
                                           nt * RPT + ky + RPT, kx:kx + W]

                    for oc in (0, 1):
                        last = (ii == len(imgs) - 1 and nt == NTILES - 1
                                and oc == 1)
                        if not last:
                            ps = pspool.tile([128, NFREE], f32,
                                             tag=f"ps{oc}")
                            for ti in range(9):
                                ky, kx = divmod(ti, 3)
                                nc.tensor.matmul(
                                    ps[:, :], wslot(10 * oc + ti),
                                    rhs(oc, ky, kx),
                                    start=(ti == 0), stop=False,
                                )
                            # cross-chunk center-tap W1
                            nc.tensor.matmul(
                                ps[:, :], wslot(10 * oc + 9),
                                rhs(1 - oc, 1, 1),
                                start=False, stop=True,
                            )
                            ot = opool.tile([128, NFREE], bf16,
                                            tag=f"ot{oc}")
                            nc.vector.tensor_copy(ot[:, :], ps[:, :])
                            eng = nc.sync if oc == 0 else nc.scalar
                            eng.dma_start(
                                out=out_r[img, 2 * oc:2 * oc + 2, :,
                                          nt * RPT:(nt + 1) * RPT, :],
                                in_=ot[:, :],
                            )
                        else:
                            # final tile: one full-width copy + one DMA on
                            # the low-latency SP queue. With bf16 outputs
                            # the transfer is small; a second HWDGE+DGE
                            # chain costs more than the bigger copy saves.
                            ps = pspool.tile([128, NFREE], f32,
                                             tag=f"ps{oc}", name="psl")
                            for ti in range(9):
                                ky, kx = divmod(ti, 3)
                                nc.tensor.matmul(
                                    ps[:, :], wslot(10 * oc + ti),
                                    rhs(oc, ky, kx),
                                    start=(ti == 0), stop=False,
                                )
                            nc.tensor.matmul(
                                ps[:, :], wslot(10 * oc + 9),
                                rhs(1 - oc, 1, 1),
                                start=False, stop=True,
                            )
                            ot = opool.tile([128, NFREE], bf16,
                                            tag=f"ot{oc}", name="otl")
                            nc.vector.tensor_copy(ot[:, :], ps[:, :])
                            nc.sync.dma_start(
                                out=out_r[img, 2 * oc:2 * oc + 2, :,
                                          nt * RPT:(nt + 1) * RPT, :],
                                in_=ot[:, :],
                            )

    nc.compile()
    return nc


def _get_prog():
    global _PROG
    if _PROG is None:
        _PROG = _build()
    return _PROG


def _prep_weights(Wk, W1):
    idx = [np.arange(g, 256, 4) for g in range(4)]
    wslabs = np.zeros((NSLOTS, 128, 128), np.float32)
    for c in (0, 1):
        gs = (2 * c, 2 * c + 1)
        for ti in range(9):
            ky, kx = divmod(ti, 3)
            s = 10 * c + ti
            for a in (0, 1):        # ic block position
                for b in (0, 1):    # oc block position
                    ga, gb = gs[a], gs[b]
                    if a == b:
                        blk = Wk[np.ix_(idx[gb], idx[ga])][:, :, ky, kx].T
                    elif ti == 4:
                        blk = W1[np.ix_(idx[gb], idx[ga])].T
                    else:
                        continue
                    wslabs[s, 64 * a:64 * a + 64, 64 * b:64 * b + 64] = blk
        # cross-chunk center-tap W1: other ic chunk -> oc chunk c
        ic_gs = (2 * (1 - c), 2 * (1 - c) + 1)
        for a, ga in enumerate(ic_gs):
            for b, gb in enumerate(gs):
                wslabs[10 * c + 9, 64 * a:64 * a + 64, 64 * b:64 * b + 64] = \
                    W1[np.ix_(idx[gb], idx[ga])].T
    # SBUF layout [K partition, slot*128 + m]
    return np.ascontiguousarray(
        wslabs.transpose(1, 0, 2).reshape(128, NSLOTS * 128))


def _make_in_maps(x, Wk, W1):
    import ml_dtypes
    bf16 = np.dtype(ml_dtypes.bfloat16)
    w_host = _prep_weights(np.asarray(Wk, np.float32),
                           np.asarray(W1, np.float32)).astype(bf16)
    xs = np.asarray(x, np.float32)
    xpad = np.zeros((B, C, HP, WP), bf16)
    xpad[:, :, 1:H + 1, 1:W + 1] = xs.astype(bf16)
    return [
        {"x": np.ascontiguousarray(xpad[i * BP:(i + 1) * BP]), "w": w_host}
        for i in range(N_CORES)
    ]


def _run(x, Wk, W1, **spmd_kwargs):
    nc = _get_prog()
    in_maps = _make_in_maps(x, Wk, W1)
    res = run_bass_kernel_spmd(nc, in_maps, list(range(N_CORES)),
                               **spmd_kwargs)
    outs = np.concatenate(
        [np.asarray(res.results[i]["out"]) for i in range(N_CORES)],
        axis=0).astype(np.float32)
    return outs, res


def kernel(x, Wk, W1):
    return _run(x, Wk, W1)[0]


# revision 38
# speedup vs baseline: 1.0065x; 1.0065x over previous
"""HetConv (3x3 block-diagonal-by-residue + 1x1 elsewhere) on 8 trn2 cores.

Strategy: data-parallel over batch (4 images/core, weights replicated).
Per core: implicit-GEMM conv over a 66x66 zero-padded SBUF image with
channels permuted by residue mod 4 (done via strided DMA access patterns,
never materialized on host). Effective weight packs into 20 [128x128]
bf16 matmul slots per spatial tile instead of 36 dense ones, in
consumption order:
  - slots 10c+ti, ti in 0..8: tap (ky,kx)=divmod(ti,3), block-diag Wk for
    groups (2c, 2c+1); center tap also carries same-chunk W1 off-diagonal.
  - slot 10c+9: cross-chunk center-tap W1 (other ic chunk -> oc chunk c).

DMA schedule is latency-tuned for the serial DMA transfer engine: inputs
stream in band-sized row strips (image 0's chunk 1 rides the ACT/HWDGE
queue interleaved with the weight DMAs; everything else on gpsimd/SWDGE)
so the PE never starves at the start or at image boundaries; outputs go
out full-width, one DMA per (band, oc chunk), split across the two HWDGE
queues; all operands are bf16 (error ~3e-3, gate 2e-2), halving DMA
traffic, with fp32 PSUM accumulation.
"""
import sys

sys.path.insert(0, "/opt/trn_rl_repo")

import numpy as np
import concourse.bacc as bacc
import concourse.mybir as mybir
from concourse import tile
from concourse.bass_utils import run_bass_kernel_spmd

N_CORES = 8
B, C, H, W = 32, 256, 64, 64
BP = B // N_CORES          # images per core
HP, WP = H + 2, W + 2      # padded image
NTILES = 8                 # output row-bands per image
RPT = H // NTILES          # rows per band
NFREE = RPT * W            # matmul moving free size (512)
NSLOTS = 20

_PROG = None


def _build(reps=1):
    nc = bacc.Bacc("TRN2", target_bir_lowering=False, debug=False,
                   num_devices=N_CORES)
    f32 = mybir.dt.float32
    bf16 = mybir.dt.bfloat16

    # x arrives host-padded to [BP, C, 66, 66] (zero border), pre-cast to
    # bf16 (error ~3e-3 vs the 2e-2 gate; halves all DMA traffic).
    x = nc.dram_tensor("x", [BP, C, HP, WP], bf16, kind="ExternalInput").ap()
    w = nc.dram_tensor("w", [128, NSLOTS * 128], bf16,
                       kind="ExternalInput").ap()
    out = nc.dram_tensor("out", [BP, C, H, W], bf16,
                         kind="ExternalOutput").ap()

    # channel c = 4k + g  ->  [b, g, k, ...]
    x_r = x.rearrange("b (k four) h w -> b four k h w", four=4)
    out_r = out.rearrange("b (k four) h w -> b four k h w", four=4)

    # input row strips: band 0 needs padded rows 0..9; band nt (>=1) needs
    # rows 8nt..8nt+9 = tail of strip nt-1 plus strip nt
    strips = [(0, 10)] + [(8 * k + 2, 8) for k in range(1, NTILES)]

    with tile.TileContext(nc) as tc:
        with (
            tc.tile_pool(name="wpool", bufs=1) as wpool,
            tc.tile_pool(name="xpool", bufs=2) as xpool,
            tc.tile_pool(name="opool", bufs=4) as opool,
            tc.tile_pool(name="pspool", bufs=4, space="PSUM") as pspool,
        ):
            wt = wpool.tile([128, NSLOTS * 128], bf16)

            def wdma(eng, lo, hi):
                eng.dma_start(out=wt[:, lo * 128:hi * 128],
                              in_=w[:, lo * 128:hi * 128])

            # weight DMAs in slot-consumption order: slots 0-4 on SP so the
            # whole first tile batch is ready together (the cold-start batch
            # is charged at full speed), the rest on ACT interleaved with
            # image 0's chunk-1 strips (emitted in the loop below)
            wdma(nc.sync, 0, 5)
            wdma(nc.sync, 5, 10)

            def wslot(s):
                return wt[:, s * 128:(s + 1) * 128]

            imgs = [i % BP for i in range(BP * reps)]
            for ii, img in enumerate(imgs):
                xps = [xpool.tile([128, HP * WP], bf16, tag=f"xp{c}",
                                  name=f"xp{c}")
                       for c in (0, 1)]
                # band-interleaved strip DMAs; partitions 0-63 <- residue 2c,
                # 64-127 <- 2c+1, one contiguous run per partition per strip.
                def sdma(eng, cchunk, r0, nr):
                    eng.dma_start(
                        out=xps[cchunk][:, r0 * WP:(r0 + nr) * WP],
                        in_=x_r[img, 2 * cchunk:2 * cchunk + 2, :,
                                r0:r0 + nr, :],
                    )

                if ii == 0:
                    # image 0: chunk 0 on Pool, chunk 1 on ACT interleaved
                    # with the remaining weight DMAs, so both chunks of band
                    # 0 land in parallel and the PE ramps without starving
                    sdma(nc.scalar, 1, 0, 10)
                    wdma(nc.scalar, 10, 15)
                    wdma(nc.scalar, 15, 20)
                    sdma(nc.gpsimd, 0, 0, 8)
                    sdma(nc.gpsimd, 0, 8, 2)
                    for r0, nr in strips[1:]:
                        sdma(nc.gpsimd, 0, r0, nr)
                        sdma(nc.gpsimd, 1, r0, nr)
                else:
                    # later images stream on Pool, whose SWDGE pipeline has
                    # plenty of headroom once compute is deep
                    for r0, nr in strips:
                        for cchunk in (0, 1):
                            sdma(nc.gpsimd, cchunk, r0, nr)
                xvs = [xp[:, :].rearrange("p (h w) -> p h w", w=WP)
                       for xp in xps]

                for nt in range(NTILES):
                    def rhs(cchunk, ky, kx):
                        return xvs[cchunk][:, nt * RPT + ky:
                                           nt * RPT + ky + RPT, kx:kx + W]

                    for oc in (0, 1):
                        last = (ii == len(imgs) - 1 and nt == NTILES - 1
                                and oc == 1)
                        if not last:
                            ps = pspool.tile([128, NFREE], f32,
                                             tag=f"ps{oc}")
                            for ti in range(9):
                                ky, kx = divmod(ti, 3)
                                nc.tensor.matmul(
                                    ps[:, :], wslot(10 * oc + ti),
                                    rhs(oc, ky, kx),
                                    start=(ti == 0), stop=False,
                                )
                            # cross-chunk center-tap W1
                            nc.tensor.matmul(
                                ps[:, :], wslot(10 * oc + 9),
                                rhs(1 - oc, 1, 1),
                                start=False, stop=True,
                            )
                            ot = opool.tile([128, NFREE], bf16,
                                            tag=f"ot{oc}")
                            nc.vector.tensor_copy(ot[:, :], ps[:, :])
                            eng = nc.sync if oc == 0 else nc.scalar
                            eng.dma_start(
                                out=out_r[img, 2 * oc:2 * oc + 2, :,
                                          nt * RPT:(nt + 1) * RPT, :],
                                in_=ot[:, :],
                            )
                        else:
                            # final tile: one full-width copy + one DMA on
                            # the low-latency SP queue. With bf16 outputs
                            # the transfer is small; a second HWDGE+DGE
                            # chain costs more than the bigger copy saves.
                            ps = pspool.tile([128, NFREE], f32,
                                             tag=f"ps{oc}", name="psl")
                            for ti in range(9):
                                ky, kx = divmod(ti, 3)
                                nc.tensor.matmul(
                                    ps[:, :], wslot(10 * oc + ti),
                                    rhs(oc, ky, kx),
                                    start=(ti == 0), stop=False,
                                )
                            nc.tensor.matmul(
                                ps[:, :], wslot(10 * oc + 9),
                                rhs(1 - oc, 1, 1),
                                start=False, stop=True,
                            )
                            ot = opool.tile([128, NFREE], bf16,
                                            tag=f"ot{oc}", name="otl")
                            nc.vector.tensor_copy(ot[:, :], ps[:, :])
                            nc.sync.dma_start(
                                out=out_r[img, 2 * oc:2 * oc + 2, :,
                                          nt * RPT:(nt + 1) * RPT, :],
                                in_=ot[:, :],
                            )

    nc.compile()
    return nc


def _get_prog():
    global _PROG
    if _PROG is None:
        _PROG = _build()
    return _PROG


def _prep_weights(Wk, W1):
    idx = [np.arange(g, 256, 4) for g in range(4)]
    wslabs = np.zeros((NSLOTS, 128, 128), np.float32)
    for c in (0, 1):
        gs = (2 * c, 2 * c + 1)
        for ti in range(9):
            ky, kx = divmod(ti, 3)
            s = 10 * c + ti
            for a in (0, 1):        # ic block position
                for b in (0, 1):    # oc block position
                    ga, gb = gs[a], gs[b]
                    if a == b:
                        blk = Wk[np.ix_(idx[gb], idx[ga])][:, :, ky, kx].T
                    elif ti == 4:
                        blk = W1[np.ix_(idx[gb], idx[ga])].T
                    else:
                        continue
                    wslabs[s, 64 * a:64 * a + 64, 64 * b:64 * b + 64] = blk
        # cross-chunk center-tap W1: other ic chunk -> oc chunk c
        ic_gs = (2 * (1 - c), 2 * (1 - c) + 1)
        for a, ga in enumerate(ic_gs):
            for b, gb in enumerate(gs):
                wslabs[10 * c + 9, 64 * a:64 * a + 64, 64 * b:64 * b + 64] = \
                    W1[np.ix_(idx[gb], idx[ga])].T
    # SBUF layout [K partition, slot*128 + m]
    return np.ascontiguousarray(
        wslabs.transpose(1, 0, 2).reshape(128, NSLOTS * 128))


def _make_in_maps(x, Wk, W1):
    import ml_dtypes
    bf16 = np.dtype(ml_dtypes.bfloat16)
    w_host = _prep_weights(np.asarray(Wk, np.float32),
                           np.asarray(W1, np.float32)).astype(bf16)
    xs = np.asarray(x, np.float32)
    xpad = np.zeros((B, C, HP, WP), bf16)
    xpad[:, :, 1:H + 1, 1:W + 1] = xs.astype(bf16)
    return [
        {"x": np.ascontiguousarray(xpad[i * BP:(i + 1) * BP]), "w": w_host}
        for i in range(N_CORES)
    ]


def _run(x, Wk, W1, **spmd_kwargs):
    nc = _get_prog()
    in_maps = _make_in_maps(x, Wk, W1)
    res = run_bass_kernel_spmd(nc, in_maps, list(range(N_CORES)),
                               **spmd_kwargs)
    outs = np.concatenate(
        [np.asarray(res.results[i]["out"]) for i in range(N_CORES)],
        axis=0).astype(np.float32)
    return outs, res


def kernel(x, Wk, W1):
    return _run(x, Wk, W1)[0]
